# revision 51
# baseline (speedup 1.0000x reference)
"""Decoder block Bass/Tile kernel for TRN2, SPMD over 8 cores.

Sharding: core c = (batch b = c//4, j = c%4). Each core:
  - computes K,V for ALL T_kv tokens of its batch (redundant x4, zero comm)
  - handles 512 queries: chunk A = rows [256j, 256j+256), chunk B = rows
    [256(7-j), 256(7-j)+256)  (causal load balance)
  - attention klen padded to a uniform size (1024 for A, 2048 for B); the
    causal/padding mask is a 0/1 fp16 MULTIPLY applied after exp
  - proj + residual + LN2 + MLP + residual for its 512 rows
Host gathers the 8 [512, 1024] shards into the full output.

LN1 is folded into the QKV GEMMs: the host ships RAW x^T in fp16 plus
augmented weights W_a = [W*g; -colsum(W*g); bias]. The device computes
per-token (mu, sd, rsig) from row-major x, places (mu, sd) as two rows of a
9th contraction k-tile, so  ps = x@W - mu*cs + sd*b,  and the epilogue
multiplies by rsig[t]:  rsig*(x@W) - rsig*mu*cs + b == LN(x)@W + b.
For outputs with tokens on partitions (V) the epilogue is an ACT copy with
per-partition scale; for fm outputs (K^T, Q^T, fc1-in) it is a DVE multiply
with a DRAM-broadcast rsig row.

Softmax denominators accumulate via an extra ones-column per head in V
(width 65); they are stacked into a [32,256] tile, reciprocated once, and
broadcast back through DRAM.

Matmuls fp16 with fp32 PSUM accumulation. Residual stream fp32.
"""

from contextlib import ExitStack
from dataclasses import dataclass

import numpy as np

import concourse.bass as bass
import concourse.tile as tile
from concourse import masks as cmasks
from concourse import mybir
from concourse._compat import with_exitstack

F32 = mybir.dt.float32
F16 = mybir.dt.float16


@dataclass
class Cfg:
    D: int = 1024
    DFF: int = 4096
    H: int = 16  # heads
    DH: int = 64  # head dim
    T_kv: int = 2048
    T_q: int = 512  # 2 chunks of CH
    CH: int = 256
    klenA_pad: int = 1024
    klenB_pad: int = 2048
    mmdt: str = "float16"

    @property
    def HP(self):  # head pairs
        return self.H // 2

    @property
    def VA(self):  # augmented V width (dv + ones column per head)
        return self.H * (self.DH + 1)

    @property
    def NKTA(self):
        return self.klenA_pad // 128

    @property
    def NKTB(self):
        return self.klenB_pad // 128

    @property
    def NMASK(self):  # masked k-tiles: all 8 of A + last 8 of B
        return 16


def _bcast_ap(ap, p=128):
    """[N] or [1,N] dram AP -> [p, N] with partition stride 0."""
    return bass.AP(tensor=ap.tensor, offset=ap.offset, ap=[[0, p]] + list(ap.ap)[-1:])


@with_exitstack
def decoder_kernel(ctx: ExitStack, tc: tile.TileContext, cfg: Cfg, io: dict):
    nc = tc.nc
    MD = getattr(mybir.dt, cfg.mmdt)
    D, DFF, H, DH = cfg.D, cfg.DFF, cfg.H, cfg.DH
    HP, VA, CH = cfg.HP, cfg.VA, cfg.CH
    T_kv, T_q = cfg.T_kv, cfg.T_q
    ND = D // 128  # feature tiles
    NDA = ND + 1  # + LN-augmentation tile
    NFF = DFF // 128
    NTKV = T_kv // 128
    NTQ = T_q // 128
    W2 = 2 * CH  # paired-head free width (512)

    n_vch = (VA + 511) // 512
    while VA % n_vch != 0:
        n_vch += 1
    VCH = VA // n_vch
    assert VCH <= 512

    const = ctx.enter_context(tc.tile_pool(name="const", bufs=1))
    eps_t = const.tile([128, 1], F32)
    nc.vector.memset(eps_t, 1e-5)
    ident32 = const.tile([128, 128], F32)
    cmasks.make_identity(nc, ident32)
    ident16 = const.tile([128, 128], F16)
    nc.vector.tensor_copy(ident16, ident32)

    # DRAM scratch: per-token stats rows (mu, sd, rsig) in fp16
    kv_stat = nc.dram_tensor("kv_stat", [3, T_kv], F16).ap()
    q_stat = nc.dram_tensor("q_stat", [3, T_q], F16).ap()
    q2_stat = nc.dram_tensor("q2_stat", [3, T_q], F16).ap()
    rscr = nc.dram_tensor("rscratch", [2 * HP * 2 * CH], F32).ap()

    # ---------------- persistent activations ----------------
    acts = ctx.enter_context(tc.tile_pool(name="acts", bufs=1))
    K_sb = [acts.tile([128, T_kv], MD, tag=f"K{d}", name=f"K{d}") for d in range(ND)]
    Q_sb = [acts.tile([128, 2 * T_q], MD, tag=f"Q{d}", name=f"Q{d}") for d in range(ND)]
    V_sb = [acts.tile([128, VA], MD, tag=f"V{t}", name=f"V{t}") for t in range(NTKV)]
    O_sb = [acts.tile([128, T_q], MD, tag=f"O{h}", name=f"O{h}") for h in range(HP)]
    x_q_sb = [acts.tile([128, D], F32, tag=f"xq{t}", name=f"xq{t}") for t in range(NTQ)]
    for t in range(NTQ):
        nc.scalar.dma_start(out=x_q_sb[t], in_=io["x_q"][t * 128 : (t + 1) * 128, :])

    # ---------------- stats helper ----------------
    def token_stats(x_tiles, n_tiles, stat_dram, rsig_out, pool, psum_pool,
                    ship_eng=None):
        """Per-token LN stats from row-major x tiles. Stacks fp16 (mu, sd,
        rsig) columns at cols 32*s+rt, PE-transposes once, and ships each
        stat row to stat_dram with 256B-contiguous runs."""
        stk = pool.tile([128, 96], F16, tag="ln_stk", name="ln_stk")
        for rt in range(n_tiles):
            x_t = x_tiles[rt]
            w = x_t.shape[-1]
            nsub = w // 512
            st = pool.tile([128, nsub, 6], F32, tag="ln_st")
            for s in range(nsub):
                nc.vector.bn_stats(out=st[:, s, :], in_=x_t[:, s * 512 : (s + 1) * 512])
            mv = pool.tile([128, 2], F32, tag="ln_mv")
            nc.vector.bn_aggr(out=mv, in_=st)
            sd32 = pool.tile([128, 1], F32, tag="ln_sd32")
            nc.scalar.activation(
                out=sd32, in_=mv[:, 1:2],
                func=mybir.ActivationFunctionType.Sqrt, bias=eps_t,
            )
            nc.vector.tensor_copy(stk[:, 32 + rt : 33 + rt], sd32)
            if rsig_out is not None:
                rsig = rsig_out[rt]
            else:
                rsig = pool.tile([128, 1], F32, tag="ln_rsig", name="ln_rsig")
            nc.vector.reciprocal(out=rsig, in_=sd32)
            nc.vector.tensor_copy(stk[:, rt : rt + 1], mv[:, 0:1])
            nc.vector.tensor_copy(stk[:, 64 + rt : 65 + rt], rsig)
        pst = psum_pool.tile([96, 128], F16, tag="ln_tp")
        nc.tensor.transpose(pst, stk, ident16)
        ship = ship_eng or nc.sync
        for s in range(3):
            stT = pool.tile([n_tiles, 128], F16, tag=f"ln_stT{s}", name=f"ln_stT{s}")
            nc.vector.tensor_copy(stT, pst[32 * s : 32 * s + n_tiles, :])
            ship.dma_start(
                out=stat_dram[s, :].rearrange("(t p) -> t p", p=128),
                in_=stT,
            )

    # rsig broadcast rows for K/Q epilogues (persistent)
    rbc = ctx.enter_context(tc.tile_pool(name="rbc", bufs=1))

    # ================ phase 1: stats + QKV ================
    with tc.tile_pool(name="xfm", bufs=1) as xfm, tc.tile_pool(
        name="stats", bufs=6
    ) as stp, tc.tile_pool(name="rsigkv", bufs=1) as rsp, tc.tile_pool(
        name="stps", bufs=1, space="PSUM"
    ) as stps:
        # raw x^T tiles (fp16, from host) + augmentation tile
        xT_sb = [
            xfm.tile([128, T_kv], MD, tag=f"xT{d}", name=f"xT{d}") for d in range(ND)
        ]
        for d in range(ND):
            nc.gpsimd.dma_start(out=xT_sb[d], in_=io["xT"][d * 128 : (d + 1) * 128, :])
        xT_aug = xfm.tile([128, T_kv], MD, tag="xTaug", name="xTaug")
        nc.gpsimd.memset(xT_aug, 0.0)
        xqT_sb = [
            xfm.tile([128, T_q], MD, tag=f"xqT{d}", name=f"xqT{d}") for d in range(ND)
        ]
        for d in range(ND):
            nc.scalar.dma_start(
                out=xqT_sb[d], in_=io["xqT"][d * 128 : (d + 1) * 128, :]
            )
        xqT_aug = xfm.tile([128, T_q], MD, tag="xqTaug", name="xqTaug")
        nc.gpsimd.memset(xqT_aug, 0.0)

        kv_rsig = [
            rsp.tile([128, 1], F32, tag=f"kvr{t}", name=f"kvr{t}")
            for t in range(NTKV)
        ]
        with tc.tile_pool(name="wqk", bufs=4) as wqk, tc.tile_pool(
            name="wv", bufs=1
        ) as wvp, tc.tile_pool(name="psqkv", bufs=6, space="PSUM") as psq:
            # preload the first Q weight tiles ahead of the memsets so their
            # DMA issues immediately
            wq_pre = []
            for do in range(4):
                wb = wqk.tile([128, NDA, 128], MD, tag="wqk")
                nc.gpsimd.dma_start(
                    out=wb,
                    in_=io["wq"][:, do * 128 : (do + 1) * 128].rearrange(
                        "(kt p) c -> p kt c", p=128
                    ),
                )
                wq_pre.append(wb)
            # Q_sb zero-fill on gpsimd (must be emitted before the Q scatter)
            for d in range(ND):
                nc.gpsimd.memset(Q_sb[d], 0.0)

            # ---- Q first: its stats need only 4 tiles, so the PE's in-order
            # queue (stats transpose -> matmuls) unblocks early. All q-stat
            # shipping stays on the scalar queue so the sync queue's head is
            # free for the x_kv loads that gate the kv stats. ----
            token_stats(x_q_sb, NTQ, q_stat, None, stp, stps, ship_eng=nc.scalar)
            nc.scalar.dma_start(out=xqT_aug[0:2, :], in_=q_stat[0:2, :])
            q_rsig_bc = rbc.tile([128, 512], F16, tag="qrbc", name="qrbc")
            nc.scalar.dma_start(out=q_rsig_bc, in_=_bcast_ap(q_stat[2:3, :]))
            xqT_all = xqT_sb + [xqT_aug]
            for do in range(ND):
                if do < 4:
                    wb = wq_pre[do]
                else:
                    wb = wqk.tile([128, NDA, 128], MD, tag="wqk")
                    nc.gpsimd.dma_start(
                        out=wb,
                        in_=io["wq"][:, do * 128 : (do + 1) * 128].rearrange(
                            "(kt p) c -> p kt c", p=128
                        ),
                    )
                ps = psq.tile([128, 512], F32, tag="psqk")
                for kt in range(NDA):
                    nc.tensor.matmul(
                        ps,
                        wb[:, kt, :],
                        xqT_all[kt],
                        start=(kt == 0),
                        stop=(kt == NDA - 1),
                    )
                # Q: scatter into per-(chunk, head) blocks with the
                # complementary head's partitions left zero
                for ci in range(2):
                    for h in range(2):
                        blk = (2 * ci + h) * CH
                        hsl = slice(h * 64, (h + 1) * 64)
                        nc.vector.tensor_mul(
                            out=Q_sb[do][hsl, blk : blk + CH],
                            in0=ps[hsl, ci * CH : (ci + 1) * CH],
                            in1=q_rsig_bc[hsl, ci * CH : (ci + 1) * CH],
                        )

            # ---- V: pre-emit the first groups' non-aug matmuls so the PE
            # chews on them while the kv stats chain runs ----
            wv_sb = [
                wvp.tile([128, VA], MD, tag=f"wv{kt}", name=f"wv{kt}")
                for kt in range(NDA)
            ]
            for kt in range(NDA):
                nc.gpsimd.dma_start(
                    out=wv_sb[kt], in_=io["wv"][kt * 128 : (kt + 1) * 128, :]
                )
            xT_all = xT_sb + [xT_aug]
            groups = [(tt, ch) for tt in range(NTKV) for ch in range(n_vch)]
            NPRE = 5
            pend = {}

            def v_partial(tt, ch):
                ps = psq.tile([128, 512], F32, tag="psqk")
                for kt in range(ND):
                    nc.tensor.matmul(
                        ps[:, 0:VCH],
                        xT_all[kt][:, tt * 128 : (tt + 1) * 128],
                        wv_sb[kt][:, ch * VCH : (ch + 1) * VCH],
                        start=(kt == 0),
                        stop=False,
                    )
                return ps

            def v_finish(tt, ch, ps):
                nc.tensor.matmul(
                    ps[:, 0:VCH],
                    xT_aug[:, tt * 128 : (tt + 1) * 128],
                    wv_sb[ND][:, ch * VCH : (ch + 1) * VCH],
                    start=False,
                    stop=True,
                )
                nc.scalar.activation(
                    out=V_sb[tt][:, ch * VCH : (ch + 1) * VCH],
                    in_=ps[:, 0:VCH],
                    func=mybir.ActivationFunctionType.Copy,
                    scale=kv_rsig[tt],
                )

            for tt, ch in groups[:NPRE]:
                pend[(tt, ch)] = v_partial(tt, ch)

            # ---- kv stats (16 tiles) ----
            with tc.tile_pool(name="xkv_rm", bufs=8) as xrm:
                xkv_tiles = []
                for rt in range(NTKV):
                    x_t = xrm.tile([128, D], F16, tag="xkv_in")
                    nc.sync.dma_start(
                        out=x_t, in_=io["x_kv"][rt * 128 : (rt + 1) * 128, :]
                    )
                    xkv_tiles.append(x_t)
                token_stats(xkv_tiles, NTKV, kv_stat, kv_rsig, stp, stps)
            nc.sync.dma_start(out=xT_aug[0:2, :], in_=kv_stat[0:2, :])
            kv_rsig_bc = [
                rbc.tile([128, 512], F16, tag=f"krbc{c}", name=f"krbc{c}")
                for c in range(T_kv // 512)
            ]
            for c in range(T_kv // 512):
                nc.scalar.dma_start(
                    out=kv_rsig_bc[c],
                    in_=_bcast_ap(kv_stat[2:3, c * 512 : (c + 1) * 512]),
                )

            # ---- rest of V ----
            for tt, ch in groups:
                if (tt, ch) in pend:
                    v_finish(tt, ch, pend[(tt, ch)])
                else:
                    ps = v_partial(tt, ch)
                    v_finish(tt, ch, ps)

            # ---- K (weights stationary, fm out; epilogue = DVE mult by
            # rsig broadcast row) ----
            for do in range(ND):
                wb = wqk.tile([128, NDA, 128], MD, tag="wqk")
                nc.gpsimd.dma_start(
                    out=wb,
                    in_=io["wk"][:, do * 128 : (do + 1) * 128].rearrange(
                        "(kt p) c -> p kt c", p=128
                    ),
                )
                for tch in range(T_kv // 512):
                    ps = psq.tile([128, 512], F32, tag="psqk")
                    for kt in range(NDA):
                        nc.tensor.matmul(
                            ps,
                            wb[:, kt, :],
                            xT_all[kt][:, tch * 512 : (tch + 1) * 512],
                            start=(kt == 0),
                            stop=(kt == NDA - 1),
                        )
                    nc.vector.tensor_mul(
                        out=K_sb[do][:, tch * 512 : (tch + 1) * 512],
                        in0=ps,
                        in1=kv_rsig_bc[tch],
                    )

    # ---------------- attention + proj ----------------
    mid = ctx.enter_context(tc.tile_pool(name="mid", bufs=1))
    x2_sb = [mid.tile([128, D], F32, tag=f"x2_{t}", name=f"x2_{t}") for t in range(NTQ)]
    xq2_fm = [
        mid.tile([128, T_q], MD, tag=f"xq2fm{d}", name=f"xq2fm{d}") for d in range(ND)
    ]
    # softmax denominators: slot s lives at partition 32*(s%4), cols
    # (s//4)*CH; DMA'd to rscr as 4 rows of 2048
    den4 = mid.tile([128, 2 * HP * 2 * CH // 4], F32, tag="den4", name="den4")
    chunks = [(0, cfg.NKTA, 0), (1, cfg.NKTB, cfg.NKTA - 8)]  # (ci, nkt, mask_off)
    with tc.tile_pool(name="attn_w", bufs=1) as awp:
        wproj_sb = [
            awp.tile([128, D], MD, tag=f"wp{d}", name=f"wp{d}") for d in range(ND)
        ]
        with tc.tile_pool(name="attn_m", bufs=1) as mp, tc.tile_pool(
            name="attn_p", bufs=6
        ) as pp, tc.tile_pool(name="attn_ps", bufs=4, space="PSUM"
        ) as aps, tc.tile_pool(name="attn_po", bufs=4, space="PSUM"
        ) as ops:
            # masks first (chunk A needs them immediately); wproj is not
            # consumed until the first chunk's projection
            masks = []
            for k in range(cfg.NMASK):
                m = mp.tile([128, W2], MD, tag=f"mask{k}", name=f"mask{k}")
                nc.gpsimd.dma_start(out=m, in_=io["masks"][k, :, :])
                masks.append(m)
            for d in range(ND):
                nc.gpsimd.dma_start(
                    out=wproj_sb[d], in_=io["wproj"][d * 128 : (d + 1) * 128, :]
                )
            # den slot layout: chunk ci owns rows {64ci, 64ci+32}; within a
            # chunk, s = 2*hp + h -> partition 32*(2ci + s//8), col (s%8)*CH
            for ci, nkt, moff in chunks:
                cc = slice(ci * CH, (ci + 1) * CH)
                for hp in range(HP):
                    po = [
                        ops.tile([128, CH], F32, tag="po", name="po") for _ in range(2)
                    ]
                    for kti in range(nkt):
                        ps = aps.tile([128, W2], F32, tag="ps_s")
                        kcol = slice(kti * 128, (kti + 1) * 128)
                        nc.tensor.matmul(
                            ps,
                            K_sb[hp][:, kcol],
                            Q_sb[hp][:, ci * W2 : (ci + 1) * W2],
                            start=True, stop=True,
                        )
                        pt = pp.tile([128, W2], MD, tag="pt")
                        nc.scalar.activation(
                            out=pt, in_=ps,
                            func=mybir.ActivationFunctionType.Exp,
                        )
                        if not (ci == 1 and kti < 8):
                            # gpsimd tensor_mul is ~2x slower than DVE: 3:1
                            meng = nc.gpsimd if kti % 4 == 3 else nc.vector
                            meng.tensor_mul(
                                out=pt, in0=pt, in1=masks[moff + kti]
                            )
                        for h in range(2):
                            hg = 2 * hp + h
                            nc.tensor.matmul(
                                po[h][0:65, :],
                                V_sb[kti][:, hg * 65 : hg * 65 + 65],
                                pt[:, h * CH : (h + 1) * CH],
                                start=(kti == 0),
                                stop=(kti == nkt - 1),
                            )
                    # stack denominators; evict numerators scaled by 1/4096
                    for h in range(2):
                        s = 2 * hp + h
                        dp = 32 * (2 * ci + s // 8)
                        dcol = (s % 8) * CH
                        nc.vector.tensor_copy(
                            den4[dp : dp + 1, dcol : dcol + CH],
                            po[h][64:65, :],
                        )
                        nc.vector.tensor_scalar_mul(
                            out=O_sb[hp][h * 64 : (h + 1) * 64, cc],
                            in0=po[h][0:64, :],
                            scalar1=1.0 / 4096.0,
                        )
                # ship this chunk's denominators, normalize, then project its
                # two query tiles while the other chunk's attention runs
                for r in (2 * ci, 2 * ci + 1):
                    nc.sync.dma_start(
                        out=rscr[r * 2048 : (r + 1) * 2048],
                        in_=den4[32 * r : 32 * r + 1, :],
                    )
                for hp in range(HP):
                    bc_sb = pp.tile([128, CH], F32, tag="bcsb")
                    for h in range(2):
                        s = 2 * hp + h
                        roff = (2 * ci + s // 8) * 2048 + (s % 8) * CH
                        nc.sync.dma_start(
                            out=bc_sb[h * 64 : (h + 1) * 64, :],
                            in_=bass.AP(
                                tensor=rscr.tensor,
                                offset=rscr.offset + roff,
                                ap=[[0, 64], [1, CH]],
                            ),
                        )
                    rec_bc = pp.tile([128, CH], F32, tag="recbc")
                    # table-based reciprocal on ACT (bass gates the helper on
                    # accuracy; softmax denominators tolerate ~1e-3)
                    nc.scalar.add_instruction(
                        mybir.InstActivation(
                            name=nc.get_next_instruction_name(),
                            func=mybir.ActivationFunctionType.Reciprocal,
                            ins=[
                                nc.scalar.lower_ap(bc_sb),
                                mybir.ImmediateValue(dtype=F32, value=0.0),
                                mybir.ImmediateValue(dtype=F32, value=1.0),
                                mybir.ImmediateValue(dtype=F32, value=0.0),
                            ],
                            outs=[nc.scalar.lower_ap(rec_bc)],
                        )
                    )
                    nc.vector.tensor_mul(
                        out=O_sb[hp][:, cc], in0=O_sb[hp][:, cc], in1=rec_bc
                    )
                # proj + residual for this chunk's rows (psum shares ps_s tag)
                for qt in (2 * ci, 2 * ci + 1):
                    for ch2 in range(D // 512):
                        ps = aps.tile([128, W2], F32, tag="ps_s")
                        for hp in range(ND):
                            nc.tensor.matmul(
                                ps,
                                O_sb[hp][:, qt * 128 : (qt + 1) * 128],
                                wproj_sb[hp][:, ch2 * 512 : (ch2 + 1) * 512],
                                start=(hp == 0),
                                stop=(hp == ND - 1),
                            )
                        nc.vector.tensor_add(
                            out=x2_sb[qt][:, ch2 * 512 : (ch2 + 1) * 512],
                            in0=ps,
                            in1=x_q_sb[qt][:, ch2 * 512 : (ch2 + 1) * 512],
                        )

    # ---------------- LN2 (stats + PE transpose) ----------------
    xq2T_aug = mid.tile([128, T_q], MD, tag="xq2aug", name="xq2aug")
    nc.gpsimd.memset(xq2T_aug, 0.0)
    with tc.tile_pool(name="ln2st", bufs=6) as ln2st, tc.tile_pool(
        name="tpps", bufs=4, space="PSUM"
    ) as tpps:
        token_stats(x2_sb, NTQ, q2_stat, None, ln2st, tpps, ship_eng=nc.scalar)
        nc.sync.dma_start(out=xq2T_aug[0:1, :], in_=q2_stat[0:1, :])
        nc.sync.dma_start(out=xq2T_aug[1:2, :], in_=q2_stat[1:2, :])
        for rt in range(NTQ):
            for d in range(ND):
                pst = tpps.tile([128, 128], F32, tag="tp")
                nc.tensor.transpose(
                    pst, x2_sb[rt][:, d * 128 : (d + 1) * 128], ident32
                )
                nc.scalar.copy(
                    out=xq2_fm[d][:, rt * 128 : (rt + 1) * 128], in_=pst
                )
    q2_rsig_bc = mid.tile([128, T_q], F16, tag="q2rbc", name="q2rbc")
    nc.scalar.dma_start(out=q2_rsig_bc, in_=_bcast_ap(q2_stat[2:3, :]))
    xq2_all = xq2_fm + [xq2T_aug]

    # ---------------- fc1 + gelu + fc2 (pipelined) ----------------
    ghp = ctx.enter_context(tc.tile_pool(name="gh", bufs=1))
    gh_sb = [ghp.tile([128, T_q], MD, tag=f"gh{f}", name=f"gh{f}") for f in range(NFF)]
    with tc.tile_pool(name="fc1w", bufs=4) as f1w, tc.tile_pool(
        name="fc2w", bufs=6
    ) as f2w, tc.tile_pool(name="fc2out", bufs=3) as f2o, tc.tile_pool(
        name="fcps", bufs=3, space="PSUM"
    ) as fps, tc.tile_pool(name="fc2acc", bufs=1, space="PSUM") as f2ps:
        for sweep in range(2):
            accs = {}
            for qt in range(NTQ):
                accs[qt] = f2ps.tile(
                    [128, 512], F32, tag=f"acc{qt}", name=f"acc{qt}"
                )
            for ff in range(NFF):
                if sweep == 0:
                    wb = f1w.tile([128, NDA, 128], MD, tag="wfc1")
                    nc.gpsimd.dma_start(
                        out=wb,
                        in_=io["wfc1"][:, ff * 128 : (ff + 1) * 128].rearrange(
                            "(kt p) c -> p kt c", p=128
                        ),
                    )
                    ps = fps.tile([128, T_q], F32, tag="psf1")
                    for kt in range(NDA):
                        nc.tensor.matmul(
                            ps, wb[:, kt, :], xq2_all[kt],
                            start=(kt == 0), stop=(kt == NDA - 1),
                        )
                    nc.vector.tensor_mul(out=ps, in0=ps, in1=q2_rsig_bc)
                    nc.scalar.activation(
                        out=gh_sb[ff], in_=ps,
                        func=mybir.ActivationFunctionType.Gelu,
                    )
                wb2 = f2w.tile([128, 512], MD, tag="wfc2")
                nc.gpsimd.dma_start(
                    out=wb2,
                    in_=io["wfc2"][
                        ff * 128 : (ff + 1) * 128, sweep * 512 : (sweep + 1) * 512
                    ],
                )
                for qt in range(NTQ):
                    nc.tensor.matmul(
                        accs[qt],
                        gh_sb[ff][:, qt * 128 : (qt + 1) * 128],
                        wb2,
                        start=(ff == 0),
                        stop=(ff == NFF - 1),
                    )
            for qt in range(NTQ):
                o = f2o.tile([128, 512], F32, tag="osb")
                nc.vector.tensor_add(
                    out=o,
                    in0=accs[qt],
                    in1=x2_sb[qt][:, sweep * 512 : (sweep + 1) * 512],
                )
                nc.sync.dma_start(
                    out=io["out"][
                        qt * 128 : (qt + 1) * 128,
                        sweep * 512 : (sweep + 1) * 512,
                    ],
                    in_=o,
                )


def split_drain_waits(nc):
    """walrus CoreV3 rejects >1 sync wait on several instruction types;
    split extras into single-wait NOPs preceding the instruction on the
    same (in-order) engine."""
    idx = 0

    def fix_block(b):
        nonlocal idx
        new = []
        changed = False
        for inst in b.instructions:
            si = inst.sync_info
            if si is not None and si.on_wait and len(si.on_wait) > 1:
                waits = list(si.on_wait)
                for w in waits[:-1]:
                    idx += 1
                    nop = mybir.InstNoOp(
                        name=f"I-dsplit-{idx}",
                        sync_info=mybir.SyncInfo(on_wait=[w], on_update=[]),
                    )
                    nop.engine = inst.engine
                    new.append(nop)
                inst.sync_info = mybir.SyncInfo(
                    on_wait=[waits[-1]], on_update=list(si.on_update or [])
                )
                changed = True
            new.append(inst)
        if changed:
            b.instructions = new

    for f in nc.m.functions:
        for b in f.blocks:
            fix_block(b)


def declare_io(nc, cfg: Cfg):
    c = cfg
    WD = getattr(mybir.dt, c.mmdt)
    DA = c.D + 128
    spec = {
        "x_kv": ([c.T_kv, c.D], WD, False),
        "x_q": ([c.T_q, c.D], F32, False),
        "xT": ([c.D, c.T_kv], WD, False),
        "xqT": ([c.D, c.T_q], WD, False),
        "wq": ([DA, c.D], WD, False),
        "wk": ([DA, c.D], WD, False),
        "wv": ([DA, c.VA], WD, False),
        "wproj": ([c.D, c.D], WD, False),
        "wfc1": ([DA, c.DFF], WD, False),
        "wfc2": ([c.DFF, c.D], WD, False),
        "masks": ([c.NMASK, 128, 2 * c.CH], WD, False),
        "out": ([c.T_q, c.D], F32, True),
    }
    io = {}
    for name, (shape, dt, is_out) in spec.items():
        io[name] = nc.declare_dram_parameter(name, shape, dt, isOutput=is_out).ap()
    return io


def build(cfg: Cfg, split: bool = True):
    nc = bass.Bass(num_devices=8)
    io = declare_io(nc, cfg)
    with tile.TileContext(nc) as tc:
        decoder_kernel(tc, cfg, io)
    if split:
        split_drain_waits(nc)
    return nc


# ======================= host-side prep =======================


def make_masks(cfg: Cfg, qgA, qgB):
    """[16, 128, 2*CH] 0/1 multiplicative masks: chunk A k-tiles 0..7, then
    chunk B k-tiles 8..15. Duplicated for the 2 heads along the free dim."""
    m = np.zeros((cfg.NMASK, 128, 2 * cfg.CH), np.float32)
    specs = [(qgA, range(0, 8), 0), (qgB, range(8, 16), 0)]
    for qg, kis, off in specs:
        q = qg + np.arange(cfg.CH)[None, :]
        for ki in kis:
            kg = ki * 128 + np.arange(128)[:, None]
            valid = (kg <= q).astype(np.float32)
            m[off + ki, :, 0 : cfg.CH] = valid
            m[off + ki, :, cfg.CH : 2 * cfg.CH] = valid
    return m.astype(np.float16)


def _augment(w_folded, bias):
    """Stack [-colsum(w); bias] aug rows (as a 128-row tile) under w."""
    D, O = w_folded.shape
    aug = np.zeros((128, O), np.float32)
    aug[0] = -w_folded.sum(axis=0)
    aug[1] = bias
    return np.concatenate([w_folded, aug], axis=0)


def host_prep(cfg: Cfg, x, ln1_g, ln1_b, w_qkv, w_proj, ln2_g, ln2_b, w_fc1, w_fc2):
    """Returns (in_maps list of 8 dicts, assemble(results)->full out)."""
    D, H, DH = cfg.D, cfg.H, cfg.DH
    x = np.asarray(x, np.float32)
    B = x.shape[0]
    w_qkv = np.asarray(w_qkv, np.float32)
    bqkv = np.asarray(ln1_b, np.float32) @ w_qkv  # [3D]
    w_qkv = w_qkv * np.asarray(ln1_g, np.float32)[:, None]
    bq = bqkv[0:D] / np.sqrt(DH).astype(np.float32)
    bk = bqkv[D : 2 * D]
    bv = bqkv[2 * D : 3 * D]
    wq = w_qkv[:, 0:D] / np.sqrt(DH).astype(np.float32)
    wk = w_qkv[:, D : 2 * D]
    wv = w_qkv[:, 2 * D : 3 * D]
    wv_aug = np.zeros((D, cfg.VA), np.float32)
    vb_aug = np.zeros((cfg.VA,), np.float32)
    for h in range(H):
        wv_aug[:, h * (DH + 1) : h * (DH + 1) + DH] = wv[:, h * DH : (h + 1) * DH]
        vb_aug[h * (DH + 1) : h * (DH + 1) + DH] = bv[h * DH : (h + 1) * DH]
        vb_aug[h * (DH + 1) + DH] = 1.0 / 4096.0
    bfc1 = np.asarray(ln2_b, np.float32) @ np.asarray(w_fc1, np.float32)
    wfc1 = np.asarray(w_fc1, np.float32) * np.asarray(ln2_g, np.float32)[:, None]

    wd = np.float32 if cfg.mmdt == "float32" else np.float16
    weights = {
        "wq": _augment(wq, bq).astype(wd),
        "wk": _augment(wk, bk).astype(wd),
        "wv": _augment(wv_aug, vb_aug).astype(wd),
        "wproj": np.asarray(w_proj, np.float32).astype(wd),
        "wfc1": _augment(wfc1, bfc1).astype(wd),
        "wfc2": np.asarray(w_fc2, np.float32).astype(wd),
    }

    in_maps = []
    core_rows = []
    n_j = 4  # chunk pairs per batch
    for c in range(8):
        b, j = c // n_j, c % n_j
        qgA, qgB = cfg.CH * j, cfg.CH * (2 * n_j - 1 - j)
        rows = np.r_[qgA : qgA + cfg.CH, qgB : qgB + cfg.CH]
        core_rows.append((b, rows))
        im = dict(weights)
        im["x_kv"] = x[b].astype(wd)
        im["x_q"] = np.ascontiguousarray(x[b][rows])
        im["xT"] = np.ascontiguousarray(x[b].T).astype(wd)
        im["xqT"] = np.ascontiguousarray(x[b][rows].T).astype(wd)
        im["masks"] = make_masks(cfg, qgA, qgB).astype(wd)
        in_maps.append(im)

    def assemble(results):
        out = np.zeros((B, x.shape[1], D), np.float32)
        for c, (b, rows) in enumerate(core_rows):
            out[b][rows] = results[c]["out"]
        return out

    return in_maps, assemble


# ======================= public entry point =======================

LAST_RESULTS = {}
_CACHE = {}


def kernel(x, ln1_g, ln1_b, w_qkv, w_proj, ln2_g, ln2_b, w_fc1, w_fc2,
           _trace=False):
    """Full-input decoder block on 8 TRN2 NeuronCores; returns full output."""
    from concourse.bass_utils import run_bass_kernel_spmd

    cfg = Cfg()
    in_maps, assemble = host_prep(
        cfg, x, ln1_g, ln1_b, w_qkv, w_proj, ln2_g, ln2_b, w_fc1, w_fc2
    )
    if "nc" not in _CACHE:
        _CACHE["nc"] = build(cfg)
    res = run_bass_kernel_spmd(
        _CACHE["nc"], in_maps, core_ids=list(range(8)), trace=_trace
    )
    LAST_RESULTS["res"] = res
    return assemble(res.results)
